# revision 5
# baseline (speedup 1.0000x reference)
"""Trainium2 Bass kernel for BlankEmbedding (embedding lookup + blank shift-accumulate).

Reference semantics:
    out = emb[x]                               # [B, S, D]
    preblank[s] = (x[s+1]==BLANK) & (x[s]!=BLANK)   (per row; preblank[S-1]=0)
    out[s] += sum_{k=1..3} preblank[s-k] * emb[x[s-k]]   (zero-pad at row start)

Strategy: data-parallel over the 16384 flattened tokens, 2048 per core.
Each core holds the full table in DRAM and gathers its 2048 rows with
per-partition-index indirect DMAs (17 instructions: 16 token tiles of
[128, DIM] with token t = 128*i + p, plus one halo tile; SWDGE indirect
DMA is limited to 128 indices per instruction on HW and costs ~1.5us of
GPSIMD descriptor-generation each, so the count is kept minimal).

The shift-accumulate runs on-chip: per tile, s_i = w_i * g_i masks the
preblank rows (w computed on-device from the int32 token stream in one
[128,17] batch), then c_i = A.T @ s_i with a constant banded matrix
A[q,p] = 1 iff 1 <= p-q <= 3 — two 512-wide matmuls per tile, all
sharing the same stationary operand. Contributions crossing the 128-row
tile boundary come from the previous tile's last 3 masked rows via
three tiny partition-shifting SBUF->SBUF DMAs plus aligned adds. A
3-token halo tile covers runs crossing core boundaries; the halo is
blank-filled at row starts, which forces the mask to 0 there, matching
the reference zero-padding. out_i = g_i + c_i is one DVE add from PSUM,
stored with plain HWDGE DMAs.
"""

import numpy as np

VOCAB = 50257
DIM = 1024
BLANK = 100
B, S = 4, 4096
N_CORES = 8
TOK = B * S                  # 16384 flattened tokens
TPC = TOK // N_CORES         # 2048 tokens per core
P = 128                      # SBUF partitions
NT = TPC // P                # 16 tiles per core
HALO = 3                     # max shift distance
EXT = TPC + HALO + 1         # 2052: 3 halo + 2048 tokens + 1 pad
NMM = DIM // 512             # matmul free-dim chunks per tile

_CACHE = {}


def _shift_const():
    """A[q,p] = 1 iff 1 <= p-q <= 3 (in-tile shift-accumulate matrix)."""
    q = np.arange(P)[:, None]
    p = np.arange(P)[None, :]
    return ((p - q >= 1) & (p - q <= HALO)).astype(np.float32)


def _build_nc():
    from concourse import bacc, mybir, tile
    import concourse.bass as bass

    nc = bacc.Bacc(
        "TRN2", target_bir_lowering=False, debug=False, num_devices=N_CORES
    )
    i32 = mybir.dt.int32
    f32 = mybir.dt.float32
    NC = NT + 1  # tile columns incl. halo (index 0)

    idx_ext = nc.dram_tensor("idx_ext", [EXT], i32, kind="ExternalInput")
    emb = nc.dram_tensor("emb", [VOCAB, DIM], f32, kind="ExternalInput")
    a_dram = nc.dram_tensor("a_mat", [P, P], f32, kind="ExternalInput")
    out = nc.dram_tensor("out", [TPC, DIM], f32, kind="ExternalOutput")

    with tile.TileContext(nc) as tc:
        with (
            tc.tile_pool(name="sbuf", bufs=1) as pool,
            tc.tile_pool(name="psum", bufs=3, space="PSUM") as psum_pool,
        ):
            a_sb = pool.tile([P, P], f32)
            nc.scalar.dma_start(out=a_sb[:], in_=a_dram[:])

            # ---- token + next-token columns; col 0 is the halo tile ----
            ix_all = pool.tile([P, NC], i32)
            ixn_all = pool.tile([P, NC], i32)
            nc.vector.memset(ix_all[:, 0:1], 0)
            nc.vector.memset(ixn_all[:, 0:1], 0)
            nc.scalar.dma_start(
                out=ix_all[P - HALO :, 0:1], in_=idx_ext[0:HALO, None]
            )
            nc.scalar.dma_start(
                out=ixn_all[P - HALO :, 0:1], in_=idx_ext[1 : HALO + 1, None]
            )
            for i in range(NT):
                nc.scalar.dma_start(
                    out=ix_all[:, i + 1 : i + 2],
                    in_=idx_ext[HALO + P * i : HALO + P * (i + 1), None],
                )
                nc.scalar.dma_start(
                    out=ixn_all[:, i + 1 : i + 2],
                    in_=idx_ext[HALO + 1 + P * i : HALO + 1 + P * (i + 1), None],
                )

            # ---- preblank masks w = isblank(next) & ~isblank(cur), batched ----
            b_all = pool.tile([P, NC], i32)
            bn_all = pool.tile([P, NC], i32)
            w_all = pool.tile([P, NC], f32)
            nc.vector.tensor_scalar(
                out=b_all[:], in0=ix_all[:], scalar1=BLANK, scalar2=None,
                op0=mybir.AluOpType.is_equal,
            )
            nc.vector.tensor_scalar(
                out=bn_all[:], in0=ixn_all[:], scalar1=BLANK, scalar2=None,
                op0=mybir.AluOpType.is_equal,
            )
            nc.vector.tensor_scalar(  # b := 1 - b
                out=b_all[:], in0=b_all[:], scalar1=-1, scalar2=1,
                op0=mybir.AluOpType.mult, op1=mybir.AluOpType.add,
            )
            nc.vector.tensor_tensor(  # bn := bn * (1 - b)
                out=bn_all[:], in0=bn_all[:], in1=b_all[:],
                op=mybir.AluOpType.mult,
            )
            nc.vector.tensor_copy(out=w_all[:], in_=bn_all[:])

            # ---- gathers + masked rows ----
            g = []
            s = []
            for j in range(NC):
                gt = pool.tile([P, DIM], f32, name=f"g{j}", tag="g", bufs=6)
                nc.gpsimd.indirect_dma_start(
                    out=gt[:], out_offset=None, in_=emb[:],
                    in_offset=bass.IndirectOffsetOnAxis(
                        ap=ix_all[:, j : j + 1], axis=0
                    ),
                )
                g.append(gt)
                st = pool.tile([P, DIM], f32, name=f"s{j}", tag="s", bufs=4)
                nc.vector.tensor_scalar(
                    out=st[:], in0=gt[:], scalar1=w_all[:, j : j + 1],
                    scalar2=None, op0=mybir.AluOpType.mult,
                )
                s.append(st)

                if j == 0:
                    continue
                i = j - 1  # output tile index
                # in-tile corrections: c = A.T @ s
                c = psum_pool.tile([P, DIM], f32, name=f"c{i}", tag="c")
                for h in range(NMM):
                    sl = slice(512 * h, 512 * (h + 1))
                    nc.tensor.matmul(
                        out=c[:, sl], lhsT=a_sb[:], rhs=st[:, sl],
                        start=True, stop=True,
                    )
                nc.vector.tensor_tensor(  # g := g + c
                    out=gt[:], in0=gt[:], in1=c[:], op=mybir.AluOpType.add,
                )
                # cross-boundary corrections: prev tile's last d masked rows
                # land on this tile's first d partitions (aligned after a
                # partition-shifting SBUF->SBUF copy)
                for d in (1, 2, 3):
                    ed = pool.tile(
                        [HALO, DIM], f32, name=f"e{d}_{i}", tag=f"e{d}", bufs=4
                    )
                    nc.sync.dma_start(out=ed[0:d, :], in_=s[j - 1][P - d :, :])
                    nc.vector.tensor_tensor(
                        out=gt[0:d, :], in0=gt[0:d, :], in1=ed[0:d, :],
                        op=mybir.AluOpType.add,
                    )
                nc.sync.dma_start(out=out[P * i : P * (i + 1), :], in_=gt[:])

    nc.compile()
    return nc


def get_nc():
    if "nc" not in _CACHE:
        _CACHE["nc"] = _build_nc()
    return _CACHE["nc"]


def shard_inputs(x, emb_table):
    """Build per-core in_maps from full inputs."""
    flat = np.ascontiguousarray(np.asarray(x).astype(np.int32).reshape(-1))
    emb_f32 = np.ascontiguousarray(np.asarray(emb_table, dtype=np.float32))
    a_mat = _shift_const()
    in_maps = []
    for c in range(N_CORES):
        start = c * TPC
        ext = np.zeros(EXT, dtype=np.int32)
        if start % S == 0:
            # row start: blank-filled halo makes the preblank mask 0 there,
            # matching the reference's zero-padded shifts at row boundaries
            ext[:HALO] = BLANK
        else:
            ext[:HALO] = flat[start - HALO : start]
        ext[HALO : HALO + TPC] = flat[start : start + TPC]
        # ext[-1] stays 0: only read to build w at the last position, whose
        # A-matrix row is all-zero (contributions belong to the next core)
        in_maps.append({"idx_ext": ext, "emb": emb_f32, "a_mat": a_mat})
    return in_maps


def assemble_output(results):
    parts = [results[c]["out"] for c in range(N_CORES)]
    return np.concatenate(parts, axis=0).reshape(B, S, DIM)


def kernel(x, emb_table):
    from concourse.bass_utils import run_bass_kernel_spmd

    nc = get_nc()
    in_maps = shard_inputs(x, emb_table)
    res = run_bass_kernel_spmd(nc, in_maps, core_ids=list(range(N_CORES)))
    return assemble_output(res.results)


# revision 6
# speedup vs baseline: 1.3205x; 1.3205x over previous
"""Trainium2 Bass kernel for BlankEmbedding (embedding lookup + blank shift-accumulate).

Reference semantics:
    out = emb[x]                               # [B, S, D]
    preblank[s] = (x[s+1]==BLANK) & (x[s]!=BLANK)   (per row; preblank[S-1]=0)
    out[s] += sum_{k=1..3} preblank[s-k] * emb[x[s-k]]   (zero-pad at row start)

Strategy: data-parallel over the 16384 flattened tokens, 2048 per core.
Each core holds the full table in DRAM and gathers its 2048 rows with
per-partition-index indirect DMAs (17 instructions: 16 token tiles of
[128, DIM] with token t = 128*i + p, plus one halo tile; SWDGE indirect
DMA is limited to 128 indices per instruction on HW and costs ~1.5us of
GPSIMD descriptor-generation each, so the count is kept minimal).

The shift-accumulate runs on the tensor engine with base folded in:
out_i = M_i.T @ g_i + (E*w_{i-1}).T @ g_{i-1}, where M_i = I + A*w_i,
A[q,p] = 1 iff 1 <= p-q <= 3 (in-tile shifts), E[q,p] = 1 iff
1 <= p+128-q <= 3 (shifts crossing the 128-row tile boundary), and w is
the per-position preblank mask computed on-device from the int32 token
stream in one [128,17] batch. A 3-token halo tile (tile "-1") covers
runs crossing core boundaries; the halo is blank-filled at row starts,
which forces the mask to 0 there, matching the reference zero-padding.
The PSUM result is copied to SBUF on the vector engine and stored with
plain HWDGE DMAs.
"""

import numpy as np

VOCAB = 50257
DIM = 1024
BLANK = 100
B, S = 4, 4096
N_CORES = 8
TOK = B * S                  # 16384 flattened tokens
TPC = TOK // N_CORES         # 2048 tokens per core
P = 128                      # SBUF partitions
NT = TPC // P                # 16 tiles per core
HALO = 3                     # max shift distance
EXT = TPC + HALO + 1         # 2052: 3 halo + 2048 tokens + 1 pad
NMM = DIM // 512             # matmul free-dim chunks per tile

_CACHE = {}


def _shift_consts():
    """A: in-tile shift-accumulate band; E: cross-tile-boundary band; I."""
    q = np.arange(P)[:, None]
    p = np.arange(P)[None, :]
    a_mat = ((p - q >= 1) & (p - q <= HALO)).astype(np.float32)
    e_mat = ((p + P - q >= 1) & (p + P - q <= HALO)).astype(np.float32)
    i_mat = np.eye(P, dtype=np.float32)
    return a_mat, e_mat, i_mat


def _build_nc():
    from concourse import bacc, mybir, tile
    import concourse.bass as bass

    nc = bacc.Bacc(
        "TRN2", target_bir_lowering=False, debug=False, num_devices=N_CORES
    )
    i32 = mybir.dt.int32
    f32 = mybir.dt.float32
    NC = NT + 1  # tile columns incl. halo (index 0)

    idx_ext = nc.dram_tensor("idx_ext", [EXT], i32, kind="ExternalInput")
    emb = nc.dram_tensor("emb", [VOCAB, DIM], f32, kind="ExternalInput")
    a_dram = nc.dram_tensor("a_mat", [P, P], f32, kind="ExternalInput")
    e_dram = nc.dram_tensor("e_mat", [P, P], f32, kind="ExternalInput")
    i_dram = nc.dram_tensor("i_mat", [P, P], f32, kind="ExternalInput")
    out = nc.dram_tensor("out", [TPC, DIM], f32, kind="ExternalOutput")

    with tile.TileContext(nc) as tc:
        with (
            tc.tile_pool(name="sbuf", bufs=1) as pool,
            tc.tile_pool(name="psum", bufs=3, space="PSUM") as psum_pool,
        ):
            a_sb = pool.tile([P, P], f32)
            e_sb = pool.tile([P, P], f32)
            i_sb = pool.tile([P, P], f32)
            nc.scalar.dma_start(out=a_sb[:], in_=a_dram[:])
            nc.scalar.dma_start(out=e_sb[:], in_=e_dram[:])
            nc.scalar.dma_start(out=i_sb[:], in_=i_dram[:])

            # ---- token + next-token columns; col 0 is the halo tile ----
            ix_all = pool.tile([P, NC], i32)
            ixn_all = pool.tile([P, NC], i32)
            nc.vector.memset(ix_all[:, 0:1], 0)
            nc.vector.memset(ixn_all[:, 0:1], 0)
            nc.scalar.dma_start(
                out=ix_all[P - HALO :, 0:1], in_=idx_ext[0:HALO, None]
            )
            nc.scalar.dma_start(
                out=ixn_all[P - HALO :, 0:1], in_=idx_ext[1 : HALO + 1, None]
            )
            for i in range(NT):
                nc.scalar.dma_start(
                    out=ix_all[:, i + 1 : i + 2],
                    in_=idx_ext[HALO + P * i : HALO + P * (i + 1), None],
                )
                nc.scalar.dma_start(
                    out=ixn_all[:, i + 1 : i + 2],
                    in_=idx_ext[HALO + 1 + P * i : HALO + 1 + P * (i + 1), None],
                )

            # ---- preblank masks w = isblank(next) & ~isblank(cur), batched ----
            b_all = pool.tile([P, NC], i32)
            bn_all = pool.tile([P, NC], i32)
            w_all = pool.tile([P, NC], f32)
            nc.vector.tensor_scalar(
                out=b_all[:], in0=ix_all[:], scalar1=BLANK, scalar2=None,
                op0=mybir.AluOpType.is_equal,
            )
            nc.vector.tensor_scalar(
                out=bn_all[:], in0=ixn_all[:], scalar1=BLANK, scalar2=None,
                op0=mybir.AluOpType.is_equal,
            )
            nc.vector.tensor_scalar(  # b := 1 - b
                out=b_all[:], in0=b_all[:], scalar1=-1, scalar2=1,
                op0=mybir.AluOpType.mult, op1=mybir.AluOpType.add,
            )
            nc.vector.tensor_tensor(  # bn := bn * (1 - b)
                out=bn_all[:], in0=bn_all[:], in1=b_all[:],
                op=mybir.AluOpType.mult,
            )
            nc.vector.tensor_copy(out=w_all[:], in_=bn_all[:])

            # ---- gathers + per-tile matmul / copy / store ----
            g = []
            for j in range(NC):
                gt = pool.tile([P, DIM], f32, name=f"g{j}", tag="g", bufs=6)
                nc.gpsimd.indirect_dma_start(
                    out=gt[:], out_offset=None, in_=emb[:],
                    in_offset=bass.IndirectOffsetOnAxis(
                        ap=ix_all[:, j : j + 1], axis=0
                    ),
                )
                g.append(gt)
                if j == 0:
                    continue
                i = j - 1  # output tile index

                m_sb = pool.tile([P, P], f32, name=f"m{i}", tag="m", bufs=4)
                ew_sb = pool.tile([P, P], f32, name=f"ew{i}", tag="ew", bufs=4)
                nc.vector.tensor_tensor(  # M = A * w_i (bcast)
                    out=m_sb[:], in0=a_sb[:],
                    in1=w_all[:, j : j + 1].to_broadcast([P, P]),
                    op=mybir.AluOpType.mult,
                )
                nc.vector.tensor_tensor(  # M += I  (fold base into matmul)
                    out=m_sb[:], in0=m_sb[:], in1=i_sb[:],
                    op=mybir.AluOpType.add,
                )
                nc.vector.tensor_tensor(  # Ew = E * w_{i-1} (bcast)
                    out=ew_sb[:], in0=e_sb[:],
                    in1=w_all[:, j - 1 : j].to_broadcast([P, P]),
                    op=mybir.AluOpType.mult,
                )

                c = psum_pool.tile([P, DIM], f32, name=f"c{i}", tag="c")
                for h in range(NMM):
                    sl = slice(512 * h, 512 * (h + 1))
                    nc.tensor.matmul(
                        out=c[:, sl], lhsT=m_sb[:], rhs=g[j][:, sl],
                        start=True, stop=False,
                    )
                    nc.tensor.matmul(
                        out=c[:, sl], lhsT=ew_sb[:], rhs=g[j - 1][:, sl],
                        start=False, stop=True,
                    )
                o_sb = pool.tile([P, DIM], f32, name=f"o{i}", tag="o", bufs=4)
                nc.vector.tensor_copy(out=o_sb[:], in_=c[:])
                nc.sync.dma_start(out=out[P * i : P * (i + 1), :], in_=o_sb[:])

    nc.compile()
    return nc


def get_nc():
    if "nc" not in _CACHE:
        _CACHE["nc"] = _build_nc()
    return _CACHE["nc"]


def shard_inputs(x, emb_table):
    """Build per-core in_maps from full inputs."""
    flat = np.ascontiguousarray(np.asarray(x).astype(np.int32).reshape(-1))
    emb_f32 = np.ascontiguousarray(np.asarray(emb_table, dtype=np.float32))
    a_mat, e_mat, i_mat = _shift_consts()
    in_maps = []
    for c in range(N_CORES):
        start = c * TPC
        ext = np.zeros(EXT, dtype=np.int32)
        if start % S == 0:
            # row start: blank-filled halo makes the preblank mask 0 there,
            # matching the reference's zero-padded shifts at row boundaries
            ext[:HALO] = BLANK
        else:
            ext[:HALO] = flat[start - HALO : start]
        ext[HALO : HALO + TPC] = flat[start : start + TPC]
        # ext[-1] stays 0: only read to build w at the last position, whose
        # A-matrix row is all-zero (contributions belong to the next core)
        in_maps.append(
            {"idx_ext": ext, "emb": emb_f32, "a_mat": a_mat, "e_mat": e_mat,
             "i_mat": i_mat}
        )
    return in_maps


def assemble_output(results):
    parts = [results[c]["out"] for c in range(N_CORES)]
    return np.concatenate(parts, axis=0).reshape(B, S, DIM)


def kernel(x, emb_table):
    from concourse.bass_utils import run_bass_kernel_spmd

    nc = get_nc()
    in_maps = shard_inputs(x, emb_table)
    res = run_bass_kernel_spmd(nc, in_maps, core_ids=list(range(N_CORES)))
    return assemble_output(res.results)
